# revision 11
# baseline (speedup 1.0000x reference)
"""Trainium2 Bass kernel for nn_Baseline_mb_24189255811183 (gnn_message_passing).

Full on-device SPMD implementation on 8 NeuronCores.

Sharding: paths are sharded 8-ways (2048 paths/core) per the sharding hint;
link_state [L,64] and device_state [N,64] are replicated on every core; the
per-link/per-node path-state segment reductions are computed as local partial
sums and AllReduce'd each message-passing iteration; parameters replicated.

Everything lives transposed in SBUF (features on partitions):
  - pssT [65, 9*2048]  path-state sequence table (row 64 = ones for bias folding)
  - x gathers (link_state[link_to_path], device_state[node_to_path]) and the
    segment-sum gathers (pss[pl0,pl1], pss[pn0,pn1]) run on the GPSIMD engine
    via ap_gather (SBUF free-dim gather; indices int16, wrapped 16-partition
    layout, static across iterations, precomputed on host).
  - GRU gates accumulate in PSUM straight from the PE: z,r tile gets
    wx.T@x(link) + wx.T@x(node) + wh.T@h in one accumulation group; biases are
    folded via the ones-row augmentation (weights get a 65th row = bias).
  - The last iteration's link/device GRU updates are dead code and skipped.
  - Readout (relu MLP -> softplus -> capacity-weighted hop sum) fused on
    device; per-core output is the [2048] delay shard.

Host does only the cheap O(P*D) encoders and index packing (~0.1 s).
"""
import sys
sys.path.insert(0, '/opt/trn_rl_repo')
import numpy as np

P, T, L, K, N, K2, M, D = 16384, 8, 4096, 16, 2048, 32, 8, 64
ITER = 8
NCORES = 8
PLOC = P // NCORES              # 2048 paths per core
NSLOT = (T + 1) * PLOC          # 18432 pss slots (slot = t*PLOC + p)
ZSLOT = NSLOT                   # zero row for non-local segment entries
NELEM = NSLOT + 16              # padded ap_gather table size

_NC_CACHE = {}


# ---------------------------------------------------------------- host math
def _relu(v):
    return np.maximum(v, 0.0)


def _mlp2(x, w1, b1, w2, b2):
    return _relu(_relu(x @ w1 + b1) @ w2 + b2)


def _wrap_idx(idx_list):
    """int16 index list -> [64, n/16] wrapped layout for ap_gather."""
    n = idx_list.shape[0]
    w = idx_list.reshape(n // 16, 16).T.astype(np.int16)   # [16, n/16]
    return np.tile(w, (4, 1))                              # [64, n/16]


def _host_prep(inp):
    f = lambda k: np.ascontiguousarray(np.asarray(inp[k], np.float32))
    ft, fp, fps, cap = f('flow_traffic'), f('flow_packets'), f('flow_packet_size'), f('link_capacity')
    ltp, ntp = np.asarray(inp['link_to_path']), np.asarray(inp['node_to_path'])
    ptl, ptn, ltn = np.asarray(inp['path_to_link']), np.asarray(inp['path_to_node']), np.asarray(inp['link_to_node'])

    ldt = (np.asarray(inp['link_device_type']) == 0).astype(np.float32)[:, None]
    load = ft[ptl[:, :, 0], 0].sum(1)[:, None] / (cap * 1e9)
    path_state = _mlp2(np.concatenate([ft * 1e-4, fp * 1e-3, fps * 1e-3], 1),
                       f('pe_w1'), f('pe_b1'), f('pe_w2'), f('pe_b2'))
    link_state = _mlp2(np.concatenate([cap * 1e-2, load, ldt], 1),
                       f('le_w1'), f('le_b1'), f('le_w2'), f('le_b2'))
    dlm = link_state[ltn].sum(1).mean(1, keepdims=True)
    dev_enc = (np.asarray(inp['nodes']) == 0).astype(np.float32)[:, None]
    device_state = _mlp2(np.concatenate([dev_enc, dlm], 1),
                         f('de_w1'), f('de_b1'), f('de_w2'), f('de_b2'))

    aug = lambda w, b: np.ascontiguousarray(
        np.vstack([np.asarray(w, np.float32), np.asarray(b, np.float32)[None, :]]))
    pwx, pwh = aug(inp['pgru_wx'], inp['pgru_bx']), aug(inp['pgru_wh'], inp['pgru_bh'])
    lwx, lwh = aug(inp['lgru_wx'], inp['lgru_bx']), aug(inp['lgru_wh'], inp['lgru_bh'])
    dwx, dwh = aug(inp['dgru_wx'], inp['dgru_bx']), aug(inp['dgru_wh'], inp['dgru_bh'])
    w1a = aug(inp['ro_w1'], inp['ro_b1'])   # [65, 32]
    w2a = aug(inp['ro_w2'], inp['ro_b2'])   # [33, 16]
    w3a = aug(inp['ro_w3'], inp['ro_b3'])   # [17, 1]

    lstT = np.ascontiguousarray(link_state.T)     # [64, 4096]
    dstT = np.ascontiguousarray(device_state.T)   # [64, 2048]

    pl0, pl1 = ptl[:, :, 0].astype(np.int64), ptl[:, :, 1].astype(np.int64)
    pn0, pn1 = ptn[:, :, 0].astype(np.int64), ptn[:, :, 1].astype(np.int64)
    icap_all = (1.0 / cap[ltp, 0]).astype(np.float32)      # [P, T]

    in_maps = []
    for c in range(NCORES):
        lo = c * PLOC
        sl = slice(lo, lo + PLOC)
        ps0 = np.ascontiguousarray(path_state[sl].T)       # [64, 2048]
        # x gather indices, t-major order (t*PLOC + p)
        il = _wrap_idx(np.ascontiguousarray(ltp[sl].T).reshape(-1))
        inn = _wrap_idx(np.ascontiguousarray(ntp[sl].T).reshape(-1))
        # segment-sum indices over the local pss table
        gl = pl1 * PLOC + (pl0 - lo)
        gl = np.where((pl0 >= lo) & (pl0 < lo + PLOC), gl, ZSLOT)
        ipl = _wrap_idx(gl.reshape(-1))                    # [64, 4096]
        gn = pn1 * PLOC + (pn0 - lo)
        gn = np.where((pn0 >= lo) & (pn0 < lo + PLOC), gn, ZSLOT)
        ipn = _wrap_idx(gn.reshape(-1))                    # [64, 4096]
        icap = np.ascontiguousarray(icap_all[sl].T).reshape(1, T * PLOC)
        in_maps.append(dict(
            ps0=ps0, lst=lstT, dst=dstT,
            pwx=pwx, pwh=pwh, lwx=lwx, lwh=lwh, dwx=dwx, dwh=dwh,
            w1a=w1a, w2a=w2a, w3a=w3a,
            il=il, inn=inn, ipl=ipl, ipn=ipn, icap=icap,
        ))
    return in_maps


# ------------------------------------------------------------- device kernel
def _build_nc():
    import concourse.bacc as bacc
    import concourse.tile as tile
    import concourse.mybir as mybir

    f32, bf16, i16 = mybir.dt.float32, mybir.dt.bfloat16, mybir.dt.int16
    AF = mybir.ActivationFunctionType
    ALU = mybir.AluOpType
    AX = mybir.AxisListType

    nc = bacc.Bacc("TRN2", target_bir_lowering=False, debug=False,
                   num_devices=NCORES)
    dr = {}
    for name, shape, dt in (
        ('ps0', [64, PLOC], f32), ('lst', [64, L], f32), ('dst', [64, N], f32),
        ('pwx', [65, 192], f32), ('pwh', [65, 192], f32),
        ('lwx', [65, 192], f32), ('lwh', [65, 192], f32),
        ('dwx', [65, 192], f32), ('dwh', [65, 192], f32),
        ('w1a', [65, 32], f32), ('w2a', [33, 16], f32), ('w3a', [17, 1], f32),
        ('il', [64, T * PLOC // 16], i16), ('inn', [64, T * PLOC // 16], i16),
        ('ipl', [64, L * K // 16], i16), ('ipn', [64, N * K2 // 16], i16),
        ('icap', [1, T * PLOC], f32),
    ):
        dr[name] = nc.dram_tensor(name, shape, dt, kind="ExternalInput").ap()
    out_d = nc.dram_tensor("out", [1, PLOC], f32, kind="ExternalOutput").ap()

    NB = 512     # matmul free-dim chunk (one PSUM bank of f32)
    NCH = PLOC // NB

    with tile.TileContext(nc) as tc:
        with (
            tc.tile_pool(name="persist", bufs=1) as P0,
            tc.tile_pool(name="psumA", bufs=1, space="PSUM") as PA,
            tc.tile_pool(name="psumB", bufs=1, space="PSUM") as PB,
            tc.tile_pool(name="dram", bufs=1, space="DRAM") as DR,
        ):
            pssT = P0.tile([65, NELEM], f32)
            w1a_t = P0.tile([65, 32], f32)
            w2a_t = P0.tile([33, 16], f32)
            w3a_t = P0.tile([17, 1], f32)
            nc.sync.dma_start(pssT[0:64, 0:PLOC], dr['ps0'])
            nc.sync.dma_start(w1a_t[:], dr['w1a'])
            nc.sync.dma_start(w2a_t[:], dr['w2a'])
            nc.sync.dma_start(w3a_t[:], dr['w3a'])
            nc.vector.memset(pssT[64:65, :], 1.0)
            nc.vector.memset(pssT[0:64, NSLOT:NELEM], 0.0)

            def mm(out_tile, stat, mov, first, last, width=PLOC):
                """Chunked matmul: out[:, j*NB:(j+1)*NB] += stat.T @ mov[:, ...]."""
                for j in range(width // NB):
                    s = slice(j * NB, (j + 1) * NB)
                    nc.tensor.matmul(out_tile[:, s], stat, mov[:, s],
                                     start=first, stop=last)

            def gru_widths(wx_t, wh_t, Bx_aug, Bx2, Bh_aug, Hprev, Hout, z, r, u, v, scr):
                """One GRU update, transposed layout, width = PLOC columns.

                Bx_aug  [65, PLOC]: x-input with ones row (bias via wx row 64)
                Bx2     [64, PLOC] or None: second x-contribution (no bias row)
                Bh_aug  [65, PLOC]: h with ones row (bias via wh row 64)
                Hprev   [64, PLOC]: previous h (parts 0:64 of Bh_aug)
                Hout    [64, PLOC]: destination for h'
                """
                tb = PB.tile([64, PLOC], f32, tag="pb")   # hc~
                for j in range(NCH):
                    s = slice(j * NB, (j + 1) * NB)
                    nc.tensor.matmul(tb[:, s], wh_t[:, 128:192], Bh_aug[:, s],
                                     start=True, stop=True)
                nc.scalar.activation(u, tb[:], AF.Copy)
                ta = PA.tile([128, PLOC], f32, tag="pa")  # z~, r~
                for j in range(NCH):
                    s = slice(j * NB, (j + 1) * NB)
                    nc.tensor.matmul(ta[:, s], wx_t[:, 0:128], Bx_aug[:, s],
                                     start=True, stop=False)
                    if Bx2 is not None:
                        nc.tensor.matmul(ta[:, s], wx_t[0:64, 0:128], Bx2[:, s],
                                         start=False, stop=False)
                    nc.tensor.matmul(ta[:, s], wh_t[:, 0:128], Bh_aug[:, s],
                                     start=False, stop=True)
                nc.scalar.activation(z, ta[0:64, :], AF.Sigmoid)
                nc.scalar.activation(r, ta[64:128, :], AF.Sigmoid)
                tb2 = PB.tile([64, PLOC], f32, tag="pb")  # xc~
                for j in range(NCH):
                    s = slice(j * NB, (j + 1) * NB)
                    nc.tensor.matmul(tb2[:, s], wx_t[:, 128:192], Bx_aug[:, s],
                                     start=True, stop=(Bx2 is None))
                    if Bx2 is not None:
                        nc.tensor.matmul(tb2[:, s], wx_t[0:64, 128:192], Bx2[:, s],
                                         start=False, stop=True)
                nc.vector.tensor_tensor(out=v, in0=r, in1=u, op=ALU.mult)
                nc.vector.tensor_tensor(out=u, in0=tb2[:], in1=v, op=ALU.add)
                nc.scalar.activation(v, u, AF.Tanh)                       # v = c
                nc.vector.tensor_tensor(out=u, in0=Hprev, in1=v, op=ALU.subtract)
                nc.vector.tensor_tensor(out=scr, in0=z, in1=u, op=ALU.mult)
                nc.vector.tensor_tensor(out=Hout, in0=v, in1=scr, op=ALU.add)

            with tc.tile_pool(name="iter", bufs=1) as P1:
                lsA = P1.tile([65, L], f32)
                dsA = P1.tile([65, N], f32)
                ssA = P1.tile([65, L + N], f32)
                xa = P1.tile([65, PLOC], f32)
                nb = P1.tile([64, PLOC], f32)
                segb = P1.tile([64, 2 * PLOC], f32)
                zt_ = P1.tile([64, PLOC], bf16)
                rt_ = P1.tile([64, PLOC], bf16)
                u_ = segb[0:64, 0:PLOC]
                v_ = segb[0:64, PLOC:2 * PLOC]
                idxl = P1.tile([64, T * PLOC // 16], i16)
                idxn = P1.tile([64, T * PLOC // 16], i16)
                idxpl = P1.tile([64, L * K // 16], i16)
                idxpn = P1.tile([64, N * K2 // 16], i16)
                gw = {}
                for nm in ('pwx', 'pwh', 'lwx', 'lwh', 'dwx', 'dwh'):
                    gw[nm] = P1.tile([65, 192], f32, name="gw_" + nm)
                    nc.sync.dma_start(gw[nm][:], dr[nm])
                nc.sync.dma_start(lsA[0:64, :], dr['lst'])
                nc.sync.dma_start(dsA[0:64, :], dr['dst'])
                nc.sync.dma_start(idxl[:], dr['il'])
                nc.sync.dma_start(idxn[:], dr['inn'])
                nc.sync.dma_start(idxpl[:], dr['ipl'])
                nc.sync.dma_start(idxpn[:], dr['ipn'])
                nc.vector.memset(xa[64:65, :], 1.0)
                nc.vector.memset(ssA[64:65, :], 1.0)
                nc.vector.memset(lsA[64:65, :], 1.0)
                nc.vector.memset(dsA[64:65, :], 1.0)
                arb_i = DR.tile([64, L + N], f32)
                arb_o = DR.tile([64, L + N], f32)

                for it in range(ITER):
                    if it > 0:
                        nc.vector.tensor_copy(
                            pssT[0:64, 0:PLOC],
                            pssT[0:64, T * PLOC:(T + 1) * PLOC])
                    for t in range(T):
                        hs = slice(t * PLOC, (t + 1) * PLOC)
                        ns = slice((t + 1) * PLOC, (t + 2) * PLOC)
                        isl = slice(t * (PLOC // 16), (t + 1) * (PLOC // 16))
                        nc.gpsimd.ap_gather(
                            xa[0:64, :], lsA[0:64, :], idxl[:, isl],
                            channels=64, num_elems=L, d=1, num_idxs=PLOC)
                        nc.gpsimd.ap_gather(
                            nb[:], dsA[0:64, :], idxn[:, isl],
                            channels=64, num_elems=N, d=1, num_idxs=PLOC)
                        gru_widths(gw['pwx'], gw['pwh'],
                                   xa[:], nb[:], pssT[:, hs],
                                   pssT[0:64, hs], pssT[0:64, ns],
                                   zt_[:], rt_[:], u_, v_, nb[:])
                    if it == ITER - 1:
                        break  # final link/device updates are dead code
                    # segment sums over local pss -> partial SL | SN
                    for cc in range(16):
                        qs = slice(cc * 256, (cc + 1) * 256)
                        nc.gpsimd.ap_gather(
                            segb[:], pssT[0:64, :], idxpl[:, qs],
                            channels=64, num_elems=NELEM, d=1, num_idxs=4096)
                        nc.vector.reduce_sum(
                            ssA[0:64, cc * 256:(cc + 1) * 256],
                            segb[:].rearrange("p (l k) -> p l k", k=K),
                            axis=AX.X)
                    for cc in range(16):
                        qs = slice(cc * 256, (cc + 1) * 256)
                        nc.gpsimd.ap_gather(
                            segb[:], pssT[0:64, :], idxpn[:, qs],
                            channels=64, num_elems=NELEM, d=1, num_idxs=4096)
                        nc.vector.reduce_sum(
                            ssA[0:64, L + cc * 128:L + (cc + 1) * 128],
                            segb[:].rearrange("p (n k) -> p n k", k=K2),
                            axis=AX.X)
                    nc.gpsimd.dma_start(arb_i[:], ssA[0:64, :])
                    nc.gpsimd.collective_compute(
                        "AllReduce", ALU.add,
                        replica_groups=[list(range(NCORES))],
                        ins=[arb_i.opt()], outs=[arb_o.opt()])
                    nc.gpsimd.dma_start(ssA[0:64, :], arb_o[:])
                    # link GRU (2 chunks of 2048) and device GRU (1 chunk)
                    for ch in range(L // PLOC):
                        s = slice(ch * PLOC, (ch + 1) * PLOC)
                        gru_widths(gw['lwx'], gw['lwh'],
                                   ssA[:, s], None, lsA[:, s],
                                   lsA[0:64, s], lsA[0:64, s],
                                   zt_[:], rt_[:], u_, v_, nb[:])
                    gru_widths(gw['dwx'], gw['dwh'],
                               ssA[:, L:L + N], None, dsA[:, :],
                               dsA[0:64, :], dsA[0:64, :],
                               zt_[:], rt_[:], u_, v_, nb[:])

            # ---------------- readout ----------------
            with tc.tile_pool(name="ro", bufs=1) as P2:
                h1a = P2.tile([33, PLOC], f32)
                h2a = P2.tile([17, PLOC], f32)
                sp = P2.tile([1, PLOC], f32)
                e1 = P2.tile([1, PLOC], f32)
                ones1 = P2.tile([1, 1], f32)
                wt = P2.tile([1, PLOC], f32)
                da = [P2.tile([1, PLOC], f32, name="da0"),
                      P2.tile([1, PLOC], f32, name="da1")]
                icap_t = P2.tile([1, T * PLOC], f32)
                nc.sync.dma_start(icap_t[:], dr['icap'])
                nc.vector.memset(h1a[:], 1.0)
                nc.vector.memset(h2a[:], 1.0)
                nc.vector.memset(ones1[:], 1.0)
                for t in range(1, T + 1):
                    hs = slice(t * PLOC, (t + 1) * PLOC)
                    ta = PA.tile([128, PLOC], f32, tag="pa")
                    for j in range(NCH):
                        s = slice(j * NB, (j + 1) * NB)
                        nc.tensor.matmul(ta[0:32, s], w1a_t[:], pssT[:, hs][:, s],
                                         start=True, stop=True)
                    nc.scalar.activation(h1a[0:32, :], ta[0:32, :], AF.Relu)
                    tb = PB.tile([64, PLOC], f32, tag="pb")
                    for j in range(NCH):
                        s = slice(j * NB, (j + 1) * NB)
                        nc.tensor.matmul(tb[0:16, s], w2a_t[:], h1a[:, s],
                                         start=True, stop=True)
                    nc.scalar.activation(h2a[0:16, :], tb[0:16, :], AF.Relu)
                    ta2 = PA.tile([128, PLOC], f32, tag="pa")
                    for j in range(NCH):
                        s = slice(j * NB, (j + 1) * NB)
                        nc.tensor.matmul(ta2[0:1, s], w3a_t[:], h2a[:, s],
                                         start=True, stop=True)
                    # softplus(x) = relu(x) + ln(1 + exp(-|x|))
                    nc.scalar.activation(e1[:], ta2[0:1, :], AF.Abs)
                    nc.scalar.activation(sp[:], e1[:], AF.Exp, scale=-1.0)
                    nc.scalar.activation(e1[:], sp[:], AF.Ln, bias=ones1[:])
                    nc.scalar.activation(sp[:], ta2[0:1, :], AF.Relu)
                    nc.vector.tensor_tensor(out=wt[:], in0=sp[:], in1=e1[:],
                                            op=ALU.add)
                    nc.vector.tensor_tensor(
                        out=e1[:], in0=wt[:],
                        in1=icap_t[:, (t - 1) * PLOC:t * PLOC], op=ALU.mult)
                    if t == 1:
                        nc.vector.tensor_copy(da[1][:], e1[:])
                    else:
                        nc.vector.tensor_tensor(out=da[t % 2][:], in0=da[1 - t % 2][:],
                                                in1=e1[:], op=ALU.add)
                nc.sync.dma_start(out_d, da[T % 2][:])
    nc.compile()
    return nc


def kernel(**inputs):
    from concourse.bass_utils import run_bass_kernel_spmd

    in_maps = _host_prep(inputs)
    if "nc" not in _NC_CACHE:
        _NC_CACHE["nc"] = _build_nc()
    nc = _NC_CACHE["nc"]
    res = run_bass_kernel_spmd(nc, in_maps, core_ids=list(range(NCORES)))
    full = np.empty((P, 1), np.float32)
    for c in range(NCORES):
        full[c * PLOC:(c + 1) * PLOC, 0] = np.asarray(res.results[c]["out"])[0]
    kernel._last_res = res
    return full
